# revision 1
# baseline (speedup 1.0000x reference)
"""Binarized LSTM cell (BLSTMCell) Trainium2 kernel.

gates = x @ sign(W_ih).T + b_ih + hx @ sign(W_hh).T + b_hh
i, f, g, o = split(gates, 4); cy = sig(f)*cx + sig(i)*tanh(g); hy = sig(o)*tanh(cy)

Strategy: data-parallel over the batch dim (8192 -> 1024 rows/core on 8 cores).
Per core: weights are loaded fp32, sign-binarized to fp16 on ScalarE (exact: +-1
is exact in fp16 and sign() is computed from full fp32), transposed k-major via
the DMA xbar; activations are cast to fp16 on load (SWDGE) and transposed via
xbar; TensorE does the [1024x1024]@[1024x2048] gate matmul in fp16 (1 col/cycle)
accumulating fp32 in PSUM; VectorE/ScalarE run the LSTM elementwise math in fp32.
"""

import numpy as np

import concourse.bass as bass
import concourse.mybir as mybir
import concourse.tile as tile
from concourse import bass_utils

AF = mybir.ActivationFunctionType
F32 = mybir.dt.float32
F16 = mybir.dt.float16

B, IN, H = 8192, 512, 512
NCORES = 8
BL = B // NCORES  # 1024 rows per core
NMT = BL // 128  # 8 batch tiles per core
G = 4 * H  # 2048
KC = IN + H  # 1024 contraction
NKT = KC // 128  # 8 k-tiles
NGT = G // 128  # 16 g-tiles


def _split_waits(nc, max_waits=1):
    """The public neuronxcc walrus rejects instructions with >1 SyncWait.
    Split excess waits into standalone EventSemaphore insts on the same engine
    immediately before the instruction (same blocking semantics)."""
    n = 0
    ctr = [0]
    for f in nc.m.functions:
        for bb in f.blocks:
            new_insts = []
            for inst in bb.instructions:
                si = inst.sync_info
                if si is not None and si.on_wait and len(si.on_wait) > max_waits:
                    waits = list(si.on_wait)
                    extra, keep = waits[:-max_waits], waits[-max_waits:]
                    for i in range(0, len(extra), max_waits):
                        ctr[0] += 1
                        new_insts.append(
                            mybir.InstEventSemaphore(
                                name=f"wsplit_{ctr[0]}",
                                opcode="EventSemaphore",
                                engine=inst.engine,
                                ins=[],
                                outs=[],
                                sync_info=mybir.SyncInfo(
                                    on_wait=list(extra[i : i + max_waits]),
                                    on_update=[],
                                ),
                                bass_nofuse=True,
                            )
                        )
                    si.on_wait = keep
                    n += 1
                new_insts.append(inst)
            bb.instructions[:] = new_insts
    return n


def build():
    nc = bass.Bass("TRN2")
    x = nc.dram_tensor("x", [BL, IN], F32, kind="ExternalInput")
    hx = nc.dram_tensor("hx", [BL, H], F32, kind="ExternalInput")
    cx = nc.dram_tensor("cx", [BL, H], F32, kind="ExternalInput")
    w_ih = nc.dram_tensor("w_ih", [G, IN], F32, kind="ExternalInput")
    w_hh = nc.dram_tensor("w_hh", [G, H], F32, kind="ExternalInput")
    b_ih = nc.dram_tensor("b_ih", [G], F32, kind="ExternalInput")
    b_hh = nc.dram_tensor("b_hh", [G], F32, kind="ExternalInput")
    hy = nc.dram_tensor("hy", [BL, H], F32, kind="ExternalOutput")
    cy = nc.dram_tensor("cy", [BL, H], F32, kind="ExternalOutput")
    bias_scratch = nc.dram_tensor("bias_scratch", [1, G], F32, kind="Internal")

    with tile.TileContext(nc) as tc:
        with (
            tc.tile_pool(name="persist", bufs=1) as persist,
            tc.tile_pool(name="wtmp", bufs=2) as wtmp,
            tc.tile_pool(name="work", bufs=3) as work,
            tc.tile_pool(name="outp", bufs=3) as outp,
            tc.tile_pool(name="psum", bufs=2, space="PSUM") as pp,
        ):
            # ---- bias: load both, add, spill, broadcast across partitions ----
            bi = persist.tile([1, G], F32)
            bh = persist.tile([1, G], F32)
            nc.scalar.dma_start(out=bi, in_=b_ih[None, :])
            nc.scalar.dma_start(out=bh, in_=b_hh[None, :])
            bsum = persist.tile([1, G], F32)
            nc.vector.tensor_add(bsum, bi, bh)
            nc.scalar.dma_start(out=bias_scratch[:, :], in_=bsum)
            bias_b = persist.tile([128, G], F32)
            nc.gpsimd.dma_start(
                out=bias_b,
                in_=bass.AP(
                    tensor=bias_scratch[:, :].tensor, offset=0, ap=[[0, 128], [1, G]]
                ),
            )

            # ---- weights: per k-tile: fp32 load -> Sign(fp16) -> xbar transpose
            # wt[kt][pk, g] = sign(Wcat[g, kt*128+pk]), Wcat = [W_ih | W_hh]
            wt = []
            for kt in range(NKT):
                src = w_ih if kt < 4 else w_hh
                koff = (kt % 4) * 128
                wn = wtmp.tile([128, NGT, 128], F32, tag="wn")
                nc.scalar.dma_start(
                    out=wn,
                    in_=src[:, koff : koff + 128].rearrange(
                        "(gt p) k -> p gt k", p=128
                    ),
                )
                ws = wtmp.tile([128, NGT, 128], F16, tag="ws")
                nc.scalar.activation(ws, wn, AF.Sign)
                wtk = persist.tile([128, G], F16, tag=f"wt{kt}")
                for gt in range(NGT):
                    nc.sync.dma_start_transpose(
                        out=wtk[:, gt * 128 : (gt + 1) * 128], in_=ws[:, gt, :]
                    )
                wt.append(wtk)

            # ---- batch tiles ----
            for m in range(NMT):
                r0 = m * 128
                xb = work.tile([128, IN], F16, tag="xb")
                hb = work.tile([128, H], F16, tag="hb")
                nc.gpsimd.dma_start(out=xb, in_=x[r0 : r0 + 128, :])
                nc.gpsimd.dma_start(out=hb, in_=hx[r0 : r0 + 128, :])
                cxt = work.tile([128, H], F32, tag="cx")
                nc.scalar.dma_start(out=cxt, in_=cx[r0 : r0 + 128, :])

                catT = work.tile([128, NKT, 128], F16, tag="catT")
                for kt in range(NKT):
                    srt = xb if kt < 4 else hb
                    koff = (kt % 4) * 128
                    nc.sync.dma_start_transpose(
                        out=catT[:, kt, :], in_=srt[:, koff : koff + 128]
                    )

                gates = pp.tile([128, G], F32)
                for kt in range(NKT):
                    for n in range(4):
                        nc.tensor.matmul(
                            gates[:, n * 512 : (n + 1) * 512],
                            catT[:, kt, :],
                            wt[kt][:, n * 512 : (n + 1) * 512],
                            start=(kt == 0),
                            stop=(kt == NKT - 1),
                        )

                def gate(i):
                    return gates[:, i * H : (i + 1) * H]

                def bias_g(i):
                    return bias_b[:, i * H : (i + 1) * H]

                a_i = work.tile([128, H], F32, tag="a_i")
                a_f = work.tile([128, H], F32, tag="a_f")
                a_g = work.tile([128, H], F32, tag="a_g")
                a_o = work.tile([128, H], F32, tag="a_o")
                nc.vector.tensor_add(a_i, gate(0), bias_g(0))
                nc.vector.tensor_add(a_f, gate(1), bias_g(1))
                nc.vector.tensor_add(a_g, gate(2), bias_g(2))
                nc.vector.tensor_add(a_o, gate(3), bias_g(3))
                s_i = work.tile([128, H], F32, tag="s_i")
                s_f = work.tile([128, H], F32, tag="s_f")
                t_g = work.tile([128, H], F32, tag="t_g")
                s_o = work.tile([128, H], F32, tag="s_o")
                nc.scalar.activation(s_i, a_i, AF.Sigmoid)
                nc.scalar.activation(s_f, a_f, AF.Sigmoid)
                nc.scalar.activation(t_g, a_g, AF.Tanh)
                nc.scalar.activation(s_o, a_o, AF.Sigmoid)
                p1 = work.tile([128, H], F32, tag="p1")
                p2 = work.tile([128, H], F32, tag="p2")
                nc.vector.tensor_mul(p1, s_f, cxt)
                nc.vector.tensor_mul(p2, s_i, t_g)
                cyt = outp.tile([128, H], F32, tag="cyt")
                nc.vector.tensor_add(cyt, p1, p2)
                t_c = work.tile([128, H], F32, tag="t_c")
                nc.scalar.activation(t_c, cyt, AF.Tanh)
                hyt = outp.tile([128, H], F32, tag="hyt")
                nc.vector.tensor_mul(hyt, s_o, t_c)
                nc.scalar.dma_start(out=cy[r0 : r0 + 128, :], in_=cyt)
                nc.scalar.dma_start(out=hy[r0 : r0 + 128, :], in_=hyt)

    _split_waits(nc)
    return nc


_NC_CACHE = []


def _get_nc():
    if not _NC_CACHE:
        _NC_CACHE.append(build())
    return _NC_CACHE[0]


def run(inputs, trace=False, tmpdir=None):
    """Shard, run on 8 cores, gather. Returns (hy, cy, BassKernelResults)."""
    x = np.ascontiguousarray(np.asarray(inputs["x"], dtype=np.float32))
    hx = np.ascontiguousarray(np.asarray(inputs["hx"], dtype=np.float32))
    cx = np.ascontiguousarray(np.asarray(inputs["cx"], dtype=np.float32))
    w_ih = np.ascontiguousarray(np.asarray(inputs["W_ih"], dtype=np.float32))
    w_hh = np.ascontiguousarray(np.asarray(inputs["W_hh"], dtype=np.float32))
    b_ih = np.ascontiguousarray(np.asarray(inputs["bias_ih"], dtype=np.float32))
    b_hh = np.ascontiguousarray(np.asarray(inputs["bias_hh"], dtype=np.float32))

    nc = _get_nc()
    in_maps = []
    for c in range(NCORES):
        sl = slice(c * BL, (c + 1) * BL)
        in_maps.append(
            {
                "x": x[sl],
                "hx": hx[sl],
                "cx": cx[sl],
                "w_ih": w_ih,
                "w_hh": w_hh,
                "b_ih": b_ih,
                "b_hh": b_hh,
            }
        )
    res = bass_utils.run_bass_kernel_spmd(
        nc, in_maps, core_ids=list(range(NCORES)), trace=trace, tmpdir=tmpdir
    )
    hy = np.concatenate([res.results[c]["hy"] for c in range(NCORES)], axis=0)
    cy = np.concatenate([res.results[c]["cy"] for c in range(NCORES)], axis=0)
    return hy, cy, res


def kernel(**inputs):
    hy, cy, _ = run(inputs, trace=False)
    return hy, cy


# revision 5
# speedup vs baseline: 2.1777x; 2.1777x over previous
"""Binarized LSTM cell (BLSTMCell) Trainium2 kernel.

gates = x @ sign(W_ih).T + b_ih + hx @ sign(W_hh).T + b_hh
i, f, g, o = split(gates, 4); cy = sig(f)*cx + sig(i)*tanh(g); hy = sig(o)*tanh(cy)

Strategy: data-parallel over the batch dim (8192 -> 1024 rows/core on 8 cores).

Per core:
- W path: the fp32 weights are never read as values. Each fp32 is viewed (AP
  bitcast) as two u16 halves; the hi half carries the sign bit. The DMA xbar
  transposes the hi halves straight DRAM->SBUF into k-major layout (strided
  2-byte source reads), then a bitwise (w & 0x8000) | 0x3C00 on-chip produces
  exact +-1.0 fp16 = sign(w) (sign bit survives any truncation/flush).
- x/hx: SWDGE cast fp32->fp16 into a DRAM scratch (chunked), then one big
  DRAM->SBUF xbar transpose per 128-row batch tile -> k-major stationary tiles.
- TensorE: [1024x1024]@[1024x2048] fp16 gate matmul (1 col/cycle), fp32 PSUM.
- VectorE adds biases and does the cy/hy products; ScalarE does sigmoid/tanh.

DMA-transposes only ever read DRAM (SBUF-sourced xbar transposes run at
~26 GB/s: ~205-byte packets; DRAM-sourced run near full rate).
"""

import numpy as np

import concourse.bass as bass
import concourse.mybir as mybir
import concourse.tile as tile
from concourse import bass_utils
from concourse.alu_op_type import AluOpType

AF = mybir.ActivationFunctionType
F32 = mybir.dt.float32
F16 = mybir.dt.float16
U16 = mybir.dt.uint16

B, IN, H = 8192, 512, 512
NCORES = 8
BL = B // NCORES  # 1024 rows per core
NMT = BL // 128  # 8 batch tiles per core
G = 4 * H  # 2048
KC = IN + H  # 1024 contraction
NKT = KC // 128  # 8 k-tiles


def _split_waits(nc, max_waits=1):
    """The public neuronxcc walrus rejects instructions with >1 SyncWait.
    Split excess waits into standalone EventSemaphore insts on the same engine
    immediately before the instruction (same blocking semantics)."""
    n = 0
    ctr = [0]
    for f in nc.m.functions:
        for bb in f.blocks:
            new_insts = []
            for inst in bb.instructions:
                si = inst.sync_info
                if si is not None and si.on_wait and len(si.on_wait) > max_waits:
                    waits = list(si.on_wait)
                    extra, keep = waits[:-max_waits], waits[-max_waits:]
                    for i in range(0, len(extra), max_waits):
                        ctr[0] += 1
                        new_insts.append(
                            mybir.InstEventSemaphore(
                                name=f"wsplit_{ctr[0]}",
                                opcode="EventSemaphore",
                                engine=inst.engine,
                                ins=[],
                                outs=[],
                                sync_info=mybir.SyncInfo(
                                    on_wait=list(extra[i : i + max_waits]),
                                    on_update=[],
                                ),
                                bass_nofuse=True,
                            )
                        )
                    si.on_wait = keep
                    n += 1
                new_insts.append(inst)
            bb.instructions[:] = new_insts
    return n


def build():
    nc = bass.Bass("TRN2")
    x = nc.dram_tensor("x", [BL, IN], F32, kind="ExternalInput")
    hx = nc.dram_tensor("hx", [BL, H], F32, kind="ExternalInput")
    cx = nc.dram_tensor("cx", [BL, H], F32, kind="ExternalInput")
    w_ih = nc.dram_tensor("w_ih", [G, IN], F32, kind="ExternalInput")
    w_hh = nc.dram_tensor("w_hh", [G, H], F32, kind="ExternalInput")
    b_ih = nc.dram_tensor("b_ih", [G], F32, kind="ExternalInput")
    b_hh = nc.dram_tensor("b_hh", [G], F32, kind="ExternalInput")
    hy = nc.dram_tensor("hy", [BL, H], F32, kind="ExternalOutput")
    cy = nc.dram_tensor("cy", [BL, H], F32, kind="ExternalOutput")
    bias_scratch = nc.dram_tensor("bias_scratch", [1, G], F32, kind="Internal")
    xh16 = nc.dram_tensor("xh16", [BL, KC], F16, kind="Internal")
    w16 = nc.dram_tensor("w16", [NKT, G, 128], F16, kind="Internal")

    with tile.TileContext(nc) as tc:
        with (
            tc.tile_pool(name="persist", bufs=1) as persist,
            tc.tile_pool(name="work", bufs=3) as work,
            tc.tile_pool(name="outp", bufs=3) as outp,
            tc.tile_pool(name="psum", bufs=2, space="PSUM") as pp,
        ):
            # ---- bias: load both, add, spill, broadcast across partitions ----
            bi = persist.tile([1, G], F32)
            bh = persist.tile([1, G], F32)
            nc.scalar.dma_start(out=bi, in_=b_ih[None, :])
            nc.scalar.dma_start(out=bh, in_=b_hh[None, :])
            bsum = persist.tile([1, G], F32)
            nc.vector.tensor_add(bsum, bi, bh)
            nc.scalar.dma_start(out=bias_scratch[:, :], in_=bsum)
            bias_b = persist.tile([128, G], F32)
            nc.gpsimd.dma_start(
                out=bias_b,
                in_=bass.AP(
                    tensor=bias_scratch[:, :].tensor, offset=0, ap=[[0, 128], [1, G]]
                ),
            )

            # sign-trick constants (per-partition u16 scalars)
            sc_signbit = persist.tile([128, 1], U16)
            sc_onehalf = persist.tile([128, 1], U16)
            nc.vector.memset(sc_signbit, 0x8000)
            nc.vector.memset(sc_onehalf, 0x3C00)  # fp16 1.0

            # ---- weights: per k-tile: SWDGE cast fp32->fp16 into DRAM scratch
            # (sign bit survives cast even when tiny values flush to +-0),
            # contiguous xbar DRAM->SBUF transpose to k-major, then
            # (w & 0x8000) | 0x3C00 -> exact +-1.0 fp16 sign(w)
            wt = persist.tile([128, NKT, G], F16)  # wt[pk, kt, g]
            for kt in range(NKT):
                src = w_ih if kt < 4 else w_hh
                koff = (kt % 4) * 128
                nc.gpsimd.dma_start(
                    out=w16[kt], in_=src[:, koff : koff + 128]
                )
                nc.sync.dma_start_transpose(out=wt[:, kt, :], in_=w16[kt])
                wu = wt[:, kt, :].bitcast(U16)
                nc.vector.tensor_scalar(
                    wu, wu, sc_signbit, sc_onehalf,
                    AluOpType.bitwise_and, AluOpType.bitwise_or,
                )

            # ---- x/hx: cast fp32->fp16 into DRAM scratch, chunked ----
            CH = 256
            for c0 in range(0, BL, CH):
                nc.gpsimd.dma_start(
                    out=xh16[c0 : c0 + CH, 0:IN], in_=x[c0 : c0 + CH, :]
                )
                nc.gpsimd.dma_start(
                    out=xh16[c0 : c0 + CH, IN:KC], in_=hx[c0 : c0 + CH, :]
                )

            # ---- batch tiles ----
            for m in range(NMT):
                r0 = m * 128
                catT = work.tile([128, NKT, 128], F16, tag="catT")
                nc.scalar.dma_start_transpose(
                    out=catT, in_=xh16[r0 : r0 + 128, :]
                )
                cxt = work.tile([128, H], F32, tag="cx")
                nc.gpsimd.dma_start(out=cxt, in_=cx[r0 : r0 + 128, :])

                gates = pp.tile([128, G], F32)
                for kt in range(NKT):
                    for n in range(4):
                        nc.tensor.matmul(
                            gates[:, n * 512 : (n + 1) * 512],
                            catT[:, kt, :],
                            wt[:, kt, n * 512 : (n + 1) * 512],
                            start=(kt == 0),
                            stop=(kt == NKT - 1),
                        )

                def gate(i):
                    return gates[:, i * H : (i + 1) * H]

                def bias_g(i):
                    return bias_b[:, i * H : (i + 1) * H]

                a_i = work.tile([128, H], F32, tag="a_i")
                a_f = work.tile([128, H], F32, tag="a_f")
                a_g = work.tile([128, H], F32, tag="a_g")
                a_o = work.tile([128, H], F32, tag="a_o")
                nc.vector.tensor_add(a_i, gate(0), bias_g(0))
                nc.vector.tensor_add(a_f, gate(1), bias_g(1))
                nc.vector.tensor_add(a_g, gate(2), bias_g(2))
                nc.vector.tensor_add(a_o, gate(3), bias_g(3))
                s_i = work.tile([128, H], F32, tag="s_i")
                s_f = work.tile([128, H], F32, tag="s_f")
                t_g = work.tile([128, H], F32, tag="t_g")
                s_o = work.tile([128, H], F32, tag="s_o")
                nc.scalar.activation(s_i, a_i, AF.Sigmoid)
                nc.scalar.activation(s_f, a_f, AF.Sigmoid)
                nc.scalar.activation(t_g, a_g, AF.Tanh)
                nc.scalar.activation(s_o, a_o, AF.Sigmoid)
                p1 = work.tile([128, H], F32, tag="p1")
                p2 = work.tile([128, H], F32, tag="p2")
                nc.vector.tensor_mul(p1, s_f, cxt)
                nc.vector.tensor_mul(p2, s_i, t_g)
                cyt = outp.tile([128, H], F32, tag="cyt")
                nc.vector.tensor_add(cyt, p1, p2)
                t_c = work.tile([128, H], F32, tag="t_c")
                nc.scalar.activation(t_c, cyt, AF.Tanh)
                hyt = outp.tile([128, H], F32, tag="hyt")
                nc.vector.tensor_mul(hyt, s_o, t_c)
                nc.scalar.dma_start(out=cy[r0 : r0 + 128, :], in_=cyt)
                nc.scalar.dma_start(out=hy[r0 : r0 + 128, :], in_=hyt)

    _split_waits(nc)
    return nc


_NC_CACHE = []


def _get_nc():
    if not _NC_CACHE:
        _NC_CACHE.append(build())
    return _NC_CACHE[0]


def run(inputs, trace=False, tmpdir=None):
    """Shard, run on 8 cores, gather. Returns (hy, cy, BassKernelResults)."""
    x = np.ascontiguousarray(np.asarray(inputs["x"], dtype=np.float32))
    hx = np.ascontiguousarray(np.asarray(inputs["hx"], dtype=np.float32))
    cx = np.ascontiguousarray(np.asarray(inputs["cx"], dtype=np.float32))
    w_ih = np.ascontiguousarray(np.asarray(inputs["W_ih"], dtype=np.float32))
    w_hh = np.ascontiguousarray(np.asarray(inputs["W_hh"], dtype=np.float32))
    b_ih = np.ascontiguousarray(np.asarray(inputs["bias_ih"], dtype=np.float32))
    b_hh = np.ascontiguousarray(np.asarray(inputs["bias_hh"], dtype=np.float32))

    nc = _get_nc()
    in_maps = []
    for c in range(NCORES):
        sl = slice(c * BL, (c + 1) * BL)
        in_maps.append(
            {
                "x": x[sl],
                "hx": hx[sl],
                "cx": cx[sl],
                "w_ih": w_ih,
                "w_hh": w_hh,
                "b_ih": b_ih,
                "b_hh": b_hh,
            }
        )
    res = bass_utils.run_bass_kernel_spmd(
        nc, in_maps, core_ids=list(range(NCORES)), trace=trace, tmpdir=tmpdir
    )
    hy = np.concatenate([res.results[c]["hy"] for c in range(NCORES)], axis=0)
    cy = np.concatenate([res.results[c]["cy"] for c in range(NCORES)], axis=0)
    return hy, cy, res


def kernel(**inputs):
    hy, cy, _ = run(inputs, trace=False)
    return hy, cy
